# revision 41
# baseline (speedup 1.0000x reference)
"""Encoder self-attention (RMSNorm + fused QKV + qk-norm + SDPA + scaled o_proj
+ residual) on 8 NeuronCores, data-parallel over the batch dim N=8.

Each core processes one batch element (L=1024 tokens, D=768, 12 heads x 64).
Host pre-transposes x / qkv_weight / o_weight (free on CPU) so the device
kernel never transposes weights; q/k are PE-transposed on chip.

Softmax exp is split between ScalarE (hardware Exp) and VectorE (Schraudolph
bit-trick exp: bf16 = bitcast(int16(x*128/ln2 + B)); the constant-factor
error cancels exactly in softmax normalization). Scores matmuls for the two
64-chan heads of a pair run concurrently via PE row-tiling. Softmax rowsums
ride as a 65th column of the AV stationary (ones column); 1/rowsum comes from
a DVE reciprocal straight out of PSUM, broadcast across the 64 head channels
by a tiny K=2 PE matmul.
"""

import numpy as np
import ml_dtypes
from contextlib import ExitStack

import concourse.bass as bass
import concourse.mybir as mybir
import concourse.tile as tile
from concourse import bacc
from concourse.bass import ts
from concourse.masks import make_identity

F32 = mybir.dt.float32
I16 = mybir.dt.int16
F8 = mybir.dt.float8e4
DR = mybir.MatmulPerfMode.DoubleRow
W_SCALE = 32.0   # host scales wT8 by this (fp8 range); v eviction divides it out
O_SCALE = 16.0   # host scales oT8 by this; phase D divides it out
P = 128
D = 768
L = 1024
NH = 12
HD = 64
TQ = L // P      # 8 token tiles
KC = D // P      # 6 contraction chunks
CT = D // P      # 6 channel tiles (q/k/v each)
NP = NH // 2     # 6 head pairs
EPS = 1e-6
AF = mybir.ActivationFunctionType
BF16 = mybir.dt.bfloat16
ALU = mybir.AluOpType

# Schraudolph exp constants for bf16 (7 mantissa bits)
EXP_K16 = 128.0 / float(np.log(2.0))
EXP_B16 = 16250.0
# Schraudolph reciprocal: bits(1/x) ~ RCP_K - hi16(f32 bits of x)
RCP_K = 32498.0


def build_bass():
    nc = bacc.Bacc(None, target_bir_lowering=False)

    x_d = nc.dram_tensor("x", [L, D], F32, kind="ExternalInput")
    xT_d = nc.dram_tensor("xT", [D, L], F8, kind="ExternalInput")
    wT_d = nc.dram_tensor("wT", [D, 3 * D], F8, kind="ExternalInput")
    oT_d = nc.dram_tensor("oT", [D, D], F8, kind="ExternalInput")
    out_d = nc.dram_tensor("out", [L, D], F32, kind="ExternalOutput")

    with tile.TileContext(nc) as tc, ExitStack() as ctx:
        persist = ctx.enter_context(tc.tile_pool(name="persist", bufs=1))
        small = ctx.enter_context(tc.tile_pool(name="small", bufs=1))

        # persistent SBUF tensors
        x_all = persist.tile([P, TQ, D], F32, tag="x_all", name="x_all")
        v_sb = [persist.tile([P, NH, HD + 1], BF16, tag=f"v{j}", name=f"v{j}")
                for j in range(TQ)]
        qnT = [persist.tile([P, L], BF16, tag=f"qnT{c}", name=f"qnT{c}") for c in range(CT)]
        knT = [persist.tile([P, L], BF16, tag=f"knT{c}", name=f"knT{c}") for c in range(CT)]
        rstd = small.tile([P, TQ], F32, tag="rstd")
        ident = small.tile([P, P], BF16, tag="ident")
        make_identity(nc, ident[:])
        eps_t = small.tile([P, 1], F32, tag="eps_t")
        nc.vector.memset(eps_t[:], EPS)
        eps64_t = small.tile([P, 1], F32, tag="eps64_t")
        nc.vector.memset(eps64_t[:], HD * EPS)
        # [1,128] selection rows for rowsum broadcast (separate tiles because
        # compute engines can only start at partition 0/32/64/96)
        selp = [small.tile([1, P], BF16, tag=f"selp{hh}", name=f"selp{hh}")
                for hh in range(2)]
        for hh in range(2):
            nc.vector.memset(selp[hh][:], 0.0)
            nc.vector.memset(selp[hh][0:1, HD * hh:HD * (hh + 1)], 1.0)
        for j in range(TQ):
            nc.vector.memset(v_sb[j][:, :, HD:HD + 1], 1.0)

        # ======================= phase A+B: rstd, QKV, qk-norm =======================
        with (
            tc.tile_pool(name="wx", bufs=1) as wx,
            tc.tile_pool(name="qk_tmp", bufs=2) as qk_tmp,
            tc.tile_pool(name="scr", bufs=2) as scr,
            tc.tile_pool(name="psB", bufs=1, space="PSUM") as psB,
            tc.tile_pool(name="psT", bufs=2, space="PSUM") as psT,
        ):
            xT_all = wx.tile([P, KC, L], F8, tag="xT_all", name="xT_all")
            wT_all = wx.tile([P, KC, 3 * D], F8, tag="wT_all", name="wT_all")
            for c in range(KC):
                nc.sync.dma_start(out=xT_all[:, c, :], in_=xT_d[ts(c, P), :])
                nc.sync.dma_start(out=wT_all[:, c, :], in_=wT_d[ts(c, P), :])

            # warm the PE HAM clock gate (4096-cycle activity window) with
            # dummy matmuls while the input DMAs land: ~5us of back-to-back
            # ident matmuls flips the clock 1.2 -> 2.4 GHz before real work
            warm = psT.tile([P, P], F32, tag="warm", bufs=1)
            for _ in range(48):
                nc.tensor.matmul(warm[:], ident[:], ident[:], start=True, stop=True)

            tn_prev = None  # (tn_q, tn_k) of tile i-1; transposed one tile late
            for i in range(TQ + 1):
                if i < TQ:
                    # per-token rstd of the input (needed for the V path only)
                    x_sb = x_all[:, i, :]
                    nc.sync.dma_start(out=x_sb, in_=x_d[ts(i, P), :])
                    sq = scr.tile([P, D], F32, tag="sq")
                    ssq = scr.tile([P, 1], F32, tag="ssq")
                    nc.scalar.activation(sq[:], x_sb, AF.Square, accum_out=ssq[:])
                    rt = scr.tile([P, 1], F32, tag="rt")
                    nc.scalar.activation(rt[:], ssq[:], AF.Sqrt, scale=1.0 / D,
                                         bias=eps_t[:])
                    nc.vector.reciprocal(rstd[:, i:i + 1], rt[:])

                # QKV matmuls (fp8 DoubleRow, d-chunk pairs) interleaved with
                # tile i-1's PE-transposes. Interleaving matters twice over:
                # it hides the norm-chain latency, and it keeps real matmuls
                # flowing (transpose-mode doesn't count as PE-busy for the
                # HAM clock gate, so a solid block of transposes would
                # re-throttle the PE to 1.2 GHz).
                tps = []
                if tn_prev is not None:
                    for t, dst in zip(tn_prev, (qnT, knT)):
                        for b in range(CT):
                            tps.append((t, dst, b))
                    tn_prev = None
                # QKV PSUM chunks packed 4x512+256 so psB fits 5 banks
                CHUNKS = ((0, 512), (512, 512), (1024, 512), (1536, 512),
                          (2048, 256))
                if i < TQ:
                    ps = [psB.tile([P, w], F32, tag=f"qkv{c}", name=f"qkvps{c}")
                          for c, (_, w) in enumerate(CHUNKS)]
                    mms = [(dp, c) for dp in range(KC // 2)
                           for c in range(len(CHUNKS))]
                else:
                    mms = []

                def emit_tp(t, dst, b):
                    tp = psT.tile([P, P], BF16, tag="tp")
                    nc.tensor.transpose(tp[:], t[:, ts(b, P)], ident[:])
                    if b % 2 == 0:
                        nc.vector.tensor_copy(dst[b][:, ts(i - 1, P)], tp[:])
                    else:
                        nc.scalar.copy(dst[b][:, ts(i - 1, P)], tp[:])

                ti = 0
                for mi, (dp, c) in enumerate(mms):
                    off, w = CHUNKS[c]
                    nc.tensor.matmul(
                        ps[c][:], xT_all[:, 2 * dp:2 * dp + 2, ts(i, P)],
                        wT_all[:, 2 * dp:2 * dp + 2, off:off + w],
                        start=(dp == 0), stop=(dp == KC // 2 - 1),
                        perf_mode=DR,
                    )
                    if mi % 5 != 4 and ti < len(tps):
                        emit_tp(*tps[ti]); ti += 1
                while ti < len(tps):
                    emit_tp(*tps[ti]); ti += 1
                    if i == TQ and ti % 2 == 0:
                        # keep HAM warm across the A->C boundary (epilogue
                        # has only transposes, which HAM ignores)
                        nc.tensor.matmul(warm[:], ident[:], ident[:],
                                         start=True, stop=True)

                if i < TQ:
                    # evictions. qk-norm is scale-invariant, so q/k skip the
                    # input rstd (eps perturbation ~1e-6 relative); v keeps it.
                    q_t = qk_tmp.tile([P, D], BF16, tag="q_t")
                    k_t = qk_tmp.tile([P, D], BF16, tag="k_t")
                    nc.vector.tensor_copy(q_t[:, 0:512], ps[0][:])
                    nc.vector.tensor_copy(q_t[:, 512:768], ps[1][:, 0:256])
                    nc.vector.tensor_copy(k_t[:, 0:256], ps[1][:, 256:512])
                    nc.vector.tensor_copy(k_t[:, 256:768], ps[2][:])
                    nc.vector.tensor_scalar(
                        v_sb[i][:, 0:8, 0:HD],
                        ps[3][:].rearrange("p (h d) -> p h d", d=HD),
                        rstd[:, i:i + 1], 1.0 / W_SCALE,
                        op0=ALU.mult, op1=ALU.mult,
                    )
                    nc.vector.tensor_scalar(
                        v_sb[i][:, 8:12, 0:HD],
                        ps[4][:].rearrange("p (h d) -> p h d", d=HD),
                        rstd[:, i:i + 1], 1.0 / W_SCALE,
                        op0=ALU.mult, op1=ALU.mult,
                    )

                    # qk-norm (RMSNorm over each head's 64 channels)
                    tn_pair = []
                    for t, isq in ((q_t, True), (k_t, False)):
                        sqg = scr.tile([P, D], F32, tag="sqg")
                        nc.scalar.activation(sqg[:], t[:], AF.Square)
                        ssg = scr.tile([P, NH, 1], F32, tag="ssg")
                        nc.vector.tensor_reduce(
                            ssg[:, :, 0],
                            sqg[:].rearrange("p (h d) -> p h d", d=HD),
                            axis=mybir.AxisListType.X,
                            op=ALU.add,
                        )
                        # q also absorbs the 1/sqrt(hd) attention scale:
                        # 1/(8*sqrt(m+eps)) = 1/sqrt(ssq + 64*eps)
                        if isq:
                            nc.scalar.activation(ssg[:], ssg[:], AF.Sqrt,
                                                 bias=eps64_t[:])
                        else:
                            nc.scalar.activation(
                                ssg[:], ssg[:], AF.Sqrt, scale=1.0 / HD,
                                bias=eps_t[:]
                            )
                        rsg = scr.tile([P, NH, 1], F32, tag="rsg")
                        nc.vector.reciprocal(rsg[:], ssg[:])
                        tn = scr.tile([P, D], BF16,
                                      tag="tn_q" if isq else "tn_k",
                                      name=f"tn_{i}_{isq}")
                        nc.gpsimd.tensor_mul(
                            tn[:].rearrange("p (h d) -> p h d", d=HD),
                            t[:].rearrange("p (h d) -> p h d", d=HD),
                            rsg[:].to_broadcast((P, NH, HD)),
                        )
                        tn_pair.append(tn)
                    tn_prev = tn_pair

        # ======================= phase C: attention =======================
        with (
            tc.tile_pool(name="oT_pool", bufs=1) as oTp,
            tc.tile_pool(name="attnT_pool", bufs=1) as attnp,
            tc.tile_pool(name="expT", bufs=32) as expp,
            tc.tile_pool(name="rs", bufs=1) as rsp,
            nc.allow_low_precision(reason="softmax in bf16"),
        ):
            oT_all = oTp.tile([P, CT, D], F8, tag="oT_all", name="oT_all")
            nc.sync.dma_start(
                out=oT_all[:], in_=oT_d[:].rearrange("(c p) e -> p c e", p=P)
            )
            attnT = [attnp.tile([P, L], BF16, tag=f"attnT{c}", name=f"attnT{c}")
                     for c in range(CT)]
            # normalized attention in fp8, ct-major for DoubleRow o_proj
            attnT8 = attnp.tile([P, CT, L], F8, tag="attnT8", name="attnT8")
            # 1/rowsum per head-in-pair, [1, NP, L] each (partition 0)
            rsinv = [rsp.tile([1, NP, L], BF16, tag=f"rsinv{hh}", name=f"rsinv{hh}")
                     for hh in range(2)]

            ets = {}  # (hp, hh, jt) -> exp tile [P, L] bf16

            def norm(hp):
                # broadcast 1/rowsum over the 64 chans of each head (K=1 PE
                # matmuls) and scale attnT; bc borrows a slot of the sc ring
                bc = ps_sc.tile([P, L], F32, tag="sc", name=f"bc_{hp}")
                for ic in range(2):
                    for hh in range(2):
                        nc.tensor.matmul(
                            bc[:, ts(ic, 512)], selp[hh][:],
                            rsinv[hh][:, hp, ts(ic, 512)],
                            start=(hh == 0), stop=(hh == 1),
                        )
                nc.vector.tensor_mul(attnT8[:, hp, :], attnT[hp][:], bc[:])

            # software-pipelined over head pairs: scores+exp(hp) interleaved
            # with the AV matmuls of hp-1 at jt granularity (keeps PE busy
            # while exps drain the sc ring), normalization trails by 2.
            with (
                tc.tile_pool(name="psC_sc", bufs=3, space="PSUM") as ps_sc,
                tc.tile_pool(name="psC_av", bufs=1, space="PSUM") as ps_av,
            ):
              for hp in range(NP + 2):
                do_sc = hp < NP
                do_av = 1 <= hp <= NP
                avt = {}

                def evict(hh):
                    # attn rows to ScalarE; 1/rowsum via Schraudolph bits
                    # trick (DVE): bits(1/x) ~ RCP_K - hi16(f32 x), reading
                    # the high half-words of the PSUM f32 row directly
                    off = HD * hh
                    a = avt[hh]
                    nc.scalar.copy(attnT[hp - 1][off:off + HD, :], a[0:HD, :])
                    hi16 = a[HD:HD + 1, :].bitcast(I16).rearrange(
                        "p (n two) -> p n two", two=2
                    )[:, :, 1]
                    nc.vector.tensor_scalar(
                        rsinv[hh][:, hp - 1, :].bitcast(I16),
                        hi16, -1.0, RCP_K, op0=ALU.mult, op1=ALU.add,
                    )

                for jt in range(TQ):
                    if do_av:
                        # AV group hh accumulates its 16 chunk-mms (8 k-tiles
                        # x 2 ics) over jts [4hh, 4hh+3]; a single [65,1024]
                        # tile per group keeps PSUM at 2 banks and evictions
                        # spread mid-step
                        hh = jt // 4
                        h = 2 * (hp - 1) + hh
                        if jt % 4 == 0:
                            if hh == 1:
                                evict(0)  # free the single-buf av ring slot
                            avt[hh] = ps_av.tile([HD + 1, L], F32, tag="av",
                                                 name=f"av_{hp - 1}_{hh}")
                        for dj in range(2):
                            ajt = (jt % 4) * 2 + dj
                            for ic in range(2):
                                nc.tensor.matmul(
                                    avt[hh][:, ts(ic, 512)],
                                    v_sb[ajt][:, h, :],
                                    ets[(hp - 1, hh, ajt)][:, ts(ic, 512)],
                                    start=(ajt == 0), stop=(ajt == TQ - 1),
                                )
                    if do_sc:
                        sc = [ps_sc.tile([P, L], F32, tag="sc",
                                         name=f"sc_{hp}_{jt}_{hh}")
                              for hh in range(2)]
                        for ic in range(2):
                            for hh in range(2):
                                off = HD * hh
                                nc.tensor.matmul(
                                    sc[hh][:, ts(ic, 512)],
                                    knT[hp][off:off + HD, ts(jt, P)],
                                    qnT[hp][off:off + HD, ts(ic, 512)],
                                    start=True, stop=True,
                                    tile_position=(off, 0),
                                )
                        for hh in range(2):
                            e = expp.tile([P, L], BF16, tag="ets",
                                          name=f"ets_{hp}_{hh}_{jt}")
                            ets[(hp, hh, jt)] = e
                            # static split tuned so ScalarE/DVE finish together
                            on_scalar = (hh == 0) or (jt == 4)
                            if on_scalar:
                                nc.scalar.activation(e[:], sc[hh][:], AF.Exp)
                            else:
                                nc.vector.tensor_scalar(
                                    e[:].bitcast(I16), sc[hh][:],
                                    EXP_K16, EXP_B16, op0=ALU.mult, op1=ALU.add,
                                )
                if do_av:
                    evict(1)
                if hp >= 2:
                    norm(hp - 2)

            # =================== phase D: o_proj + residual ===================
            with (
                tc.tile_pool(name="psD", bufs=3, space="PSUM") as psD,
                tc.tile_pool(name="outp", bufs=2) as outp,
            ):
                for i in range(TQ):
                    o0 = psD.tile([P, 512], F32, tag="o0")
                    o1 = psD.tile([P, 256], F32, tag="o1")
                    for cp in range(CT // 2):
                        lhsT = attnT8[:, 2 * cp:2 * cp + 2, ts(i, P)]
                        nc.tensor.matmul(
                            o0[:], lhsT, oT_all[:, 2 * cp:2 * cp + 2, 0:512],
                            start=(cp == 0), stop=(cp == CT // 2 - 1),
                            perf_mode=DR,
                        )
                        nc.tensor.matmul(
                            o1[:], lhsT, oT_all[:, 2 * cp:2 * cp + 2, 512:D],
                            start=(cp == 0), stop=(cp == CT // 2 - 1),
                            perf_mode=DR,
                        )
                    out_sb = outp.tile([P, D], F32, tag="out_sb")
                    nc.vector.scalar_tensor_tensor(
                        out_sb[:, 0:512], o0[:], 1.0 / O_SCALE,
                        x_all[:, i, 0:512], op0=ALU.mult, op1=ALU.add,
                    )
                    nc.vector.scalar_tensor_tensor(
                        out_sb[:, 512:D], o1[:], 1.0 / O_SCALE,
                        x_all[:, i, 512:D], op0=ALU.mult, op1=ALU.add,
                    )
                    nc.sync.dma_start(out=out_d[ts(i, P), :], in_=out_sb[:])

    nc.compile()
    return nc


_NC = None


def _get_nc():
    global _NC
    if _NC is None:
        _NC = build_bass()
    return _NC


def make_in_maps(input_NHWD, qkv_weight, o_weight, o_scale):
    N = input_NHWD.shape[0]
    f8 = ml_dtypes.float8_e4m3
    wT = np.ascontiguousarray(
        qkv_weight.reshape(3 * D, D).T.astype(np.float32) * W_SCALE
    ).astype(f8)
    oT = np.ascontiguousarray(
        (o_weight * o_scale[:, None]).T.astype(np.float32) * O_SCALE
    ).astype(f8)
    in_maps = []
    for i in range(N):
        xi = np.ascontiguousarray(input_NHWD[i].reshape(L, D).astype(np.float32))
        in_maps.append(
            {"x": xi, "xT": np.ascontiguousarray(xi.T).astype(f8),
             "wT": wT, "oT": oT}
        )
    return in_maps


def kernel(input_NHWD, qkv_weight, o_weight, o_scale):
    import time
    from concourse.bass_utils import run_bass_kernel_spmd

    input_NHWD = np.asarray(input_NHWD)
    N, H, W, _ = input_NHWD.shape
    nc = _get_nc()
    in_maps = make_in_maps(np.asarray(input_NHWD), np.asarray(qkv_weight),
                           np.asarray(o_weight), np.asarray(o_scale))
    last_err = None
    for attempt in range(3):
        try:
            res = run_bass_kernel_spmd(nc, in_maps, list(range(N)))
            out = np.stack([res.results[i]["out"] for i in range(N)], axis=0)
            return out.reshape(N, H, W, D).astype(np.float32)
        except Exception as e:  # transient device wedge: clear + retry
            last_err = e
            try:
                import jax
                jax.clear_caches()
                jax.clear_backends()
            except Exception:
                pass
            time.sleep(10)
    raise last_err


# revision 45
# speedup vs baseline: 1.0632x; 1.0632x over previous
"""Encoder self-attention (RMSNorm + fused QKV + qk-norm + SDPA + scaled o_proj
+ residual) on 8 NeuronCores, data-parallel over the batch dim N=8.

Each core processes one batch element (L=1024 tokens, D=768, 12 heads x 64).
Host pre-transposes x / qkv_weight / o_weight (free on CPU) so the device
kernel never transposes weights; q/k are PE-transposed on chip.

Softmax exp is split between ScalarE (hardware Exp) and VectorE (Schraudolph
bit-trick exp: bf16 = bitcast(int16(x*128/ln2 + B)); the constant-factor
error cancels exactly in softmax normalization). Scores matmuls for the two
64-chan heads of a pair run concurrently via PE row-tiling. Softmax rowsums
ride as a 65th column of the AV stationary (ones column); 1/rowsum comes from
a DVE reciprocal straight out of PSUM, broadcast across the 64 head channels
by a tiny K=2 PE matmul.
"""

import numpy as np
import ml_dtypes
from contextlib import ExitStack

import concourse.bass as bass
import concourse.mybir as mybir
import concourse.tile as tile
from concourse import bacc
from concourse.bass import ts
from concourse.masks import make_identity

F32 = mybir.dt.float32
I16 = mybir.dt.int16
F8 = mybir.dt.float8e4
DR = mybir.MatmulPerfMode.DoubleRow
W_SCALE = 32.0   # host scales wT8 by this (fp8 range); v eviction divides it out
O_SCALE = 16.0   # host scales oT8 by this; phase D divides it out
P = 128
D = 768
L = 1024
NH = 12
HD = 64
TQ = L // P      # 8 token tiles
KC = D // P      # 6 contraction chunks
CT = D // P      # 6 channel tiles (q/k/v each)
NP = NH // 2     # 6 head pairs
EPS = 1e-6
AF = mybir.ActivationFunctionType
BF16 = mybir.dt.bfloat16
ALU = mybir.AluOpType

# Schraudolph exp constants for bf16 (7 mantissa bits)
EXP_K16 = 128.0 / float(np.log(2.0))
EXP_B16 = 16250.0
# Schraudolph reciprocal: bits(1/x) ~ RCP_K - hi16(f32 bits of x)
RCP_K = 32498.0


def build_bass():
    nc = bacc.Bacc(None, target_bir_lowering=False)

    x_d = nc.dram_tensor("x", [L, D], F32, kind="ExternalInput")
    xT_d = nc.dram_tensor("xT", [D, L], F8, kind="ExternalInput")
    wT_d = nc.dram_tensor("wT", [D, 3 * D], F8, kind="ExternalInput")
    oT_d = nc.dram_tensor("oT", [D, D], F8, kind="ExternalInput")
    out_d = nc.dram_tensor("out", [L, D], F32, kind="ExternalOutput")

    with tile.TileContext(nc) as tc, ExitStack() as ctx:
        persist = ctx.enter_context(tc.tile_pool(name="persist", bufs=1))
        small = ctx.enter_context(tc.tile_pool(name="small", bufs=1))

        # persistent SBUF tensors
        x_all = persist.tile([P, TQ, D], F32, tag="x_all", name="x_all")
        v_sb = [persist.tile([P, NH, HD + 1], BF16, tag=f"v{j}", name=f"v{j}")
                for j in range(TQ)]
        qnT = [persist.tile([P, L], BF16, tag=f"qnT{c}", name=f"qnT{c}") for c in range(CT)]
        knT = [persist.tile([P, L], BF16, tag=f"knT{c}", name=f"knT{c}") for c in range(CT)]
        rstd = small.tile([P, TQ], F32, tag="rstd")
        ident = small.tile([P, P], BF16, tag="ident")
        make_identity(nc, ident[:])
        eps_t = small.tile([P, 1], F32, tag="eps_t")
        nc.vector.memset(eps_t[:], EPS)
        eps64_t = small.tile([P, 1], F32, tag="eps64_t")
        nc.vector.memset(eps64_t[:], HD * EPS)
        # [1,128] selection rows for rowsum broadcast (separate tiles because
        # compute engines can only start at partition 0/32/64/96)
        selp = [small.tile([1, P], BF16, tag=f"selp{hh}", name=f"selp{hh}")
                for hh in range(2)]
        for hh in range(2):
            nc.vector.memset(selp[hh][:], 0.0)
            nc.vector.memset(selp[hh][0:1, HD * hh:HD * (hh + 1)], 1.0)
        for j in range(TQ):
            nc.vector.memset(v_sb[j][:, :, HD:HD + 1], 1.0)

        # ======================= phase A+B: rstd, QKV, qk-norm =======================
        with (
            tc.tile_pool(name="wx", bufs=1) as wx,
            tc.tile_pool(name="qk_tmp", bufs=2) as qk_tmp,
            tc.tile_pool(name="scr", bufs=2) as scr,
            tc.tile_pool(name="psB", bufs=1, space="PSUM") as psB,
            tc.tile_pool(name="psT", bufs=2, space="PSUM") as psT,
        ):
            xT_all = wx.tile([P, KC, L], F8, tag="xT_all", name="xT_all")
            wT_all = wx.tile([P, KC, 3 * D], F8, tag="wT_all", name="wT_all")
            for c in range(KC):
                nc.sync.dma_start(out=xT_all[:, c, :], in_=xT_d[ts(c, P), :])
                nc.sync.dma_start(out=wT_all[:, c, :], in_=wT_d[ts(c, P), :])



            tn_prev = None  # (tn_q, tn_k) of tile i-1; transposed one tile late
            for i in range(TQ + 1):
                if i < TQ:
                    # per-token rstd of the input (needed for the V path only)
                    x_sb = x_all[:, i, :]
                    nc.sync.dma_start(out=x_sb, in_=x_d[ts(i, P), :])
                    sq = scr.tile([P, D], F32, tag="sq")
                    ssq = scr.tile([P, 1], F32, tag="ssq")
                    nc.scalar.activation(sq[:], x_sb, AF.Square, accum_out=ssq[:])
                    rt = scr.tile([P, 1], F32, tag="rt")
                    nc.scalar.activation(rt[:], ssq[:], AF.Sqrt, scale=1.0 / D,
                                         bias=eps_t[:])
                    nc.vector.reciprocal(rstd[:, i:i + 1], rt[:])

                # QKV matmuls (fp8 DoubleRow, d-chunk pairs) interleaved with
                # tile i-1's PE-transposes. Interleaving matters twice over:
                # it hides the norm-chain latency, and it keeps real matmuls
                # flowing (transpose-mode doesn't count as PE-busy for the
                # HAM clock gate, so a solid block of transposes would
                # re-throttle the PE to 1.2 GHz).
                tps = []
                if tn_prev is not None:
                    for t, dst in zip(tn_prev, (qnT, knT)):
                        for b in range(CT):
                            tps.append((t, dst, b))
                    tn_prev = None
                # QKV PSUM chunks packed 4x512+256 so psB fits 5 banks
                CHUNKS = ((0, 512), (512, 512), (1024, 512), (1536, 512),
                          (2048, 256))
                if i < TQ:
                    ps = [psB.tile([P, w], F32, tag=f"qkv{c}", name=f"qkvps{c}")
                          for c, (_, w) in enumerate(CHUNKS)]
                    mms = [(dp, c) for dp in range(KC // 2)
                           for c in range(len(CHUNKS))]
                else:
                    mms = []

                def emit_tp(t, dst, b):
                    tp = psT.tile([P, P], BF16, tag="tp")
                    nc.tensor.transpose(tp[:], t[:, ts(b, P)], ident[:])
                    if b % 2 == 0:
                        nc.vector.tensor_copy(dst[b][:, ts(i - 1, P)], tp[:])
                    else:
                        nc.scalar.copy(dst[b][:, ts(i - 1, P)], tp[:])

                # solid matmul block first (sustained PE activity flips the
                # HAM clock gate to 2.4 GHz; interleaving transposes dilutes
                # the duty cycle and keeps it cold), then the transposes
                for dp, c in mms:
                    off, w = CHUNKS[c]
                    nc.tensor.matmul(
                        ps[c][:], xT_all[:, 2 * dp:2 * dp + 2, ts(i, P)],
                        wT_all[:, 2 * dp:2 * dp + 2, off:off + w],
                        start=(dp == 0), stop=(dp == KC // 2 - 1),
                        perf_mode=DR,
                    )
                for t in tps:
                    emit_tp(*t)

                if i < TQ:
                    # evictions. qk-norm is scale-invariant, so q/k skip the
                    # input rstd (eps perturbation ~1e-6 relative); v keeps it.
                    q_t = qk_tmp.tile([P, D], BF16, tag="q_t")
                    k_t = qk_tmp.tile([P, D], BF16, tag="k_t")
                    nc.vector.tensor_copy(q_t[:, 0:512], ps[0][:])
                    nc.vector.tensor_copy(q_t[:, 512:768], ps[1][:, 0:256])
                    nc.vector.tensor_copy(k_t[:, 0:256], ps[1][:, 256:512])
                    nc.vector.tensor_copy(k_t[:, 256:768], ps[2][:])
                    nc.vector.tensor_scalar(
                        v_sb[i][:, 0:8, 0:HD],
                        ps[3][:].rearrange("p (h d) -> p h d", d=HD),
                        rstd[:, i:i + 1], 1.0 / W_SCALE,
                        op0=ALU.mult, op1=ALU.mult,
                    )
                    nc.vector.tensor_scalar(
                        v_sb[i][:, 8:12, 0:HD],
                        ps[4][:].rearrange("p (h d) -> p h d", d=HD),
                        rstd[:, i:i + 1], 1.0 / W_SCALE,
                        op0=ALU.mult, op1=ALU.mult,
                    )

                    # qk-norm (RMSNorm over each head's 64 channels)
                    tn_pair = []
                    for t, isq in ((q_t, True), (k_t, False)):
                        sqg = scr.tile([P, D], F32, tag="sqg")
                        nc.scalar.activation(sqg[:], t[:], AF.Square)
                        ssg = scr.tile([P, NH, 1], F32, tag="ssg")
                        nc.vector.tensor_reduce(
                            ssg[:, :, 0],
                            sqg[:].rearrange("p (h d) -> p h d", d=HD),
                            axis=mybir.AxisListType.X,
                            op=ALU.add,
                        )
                        # q also absorbs the 1/sqrt(hd) attention scale:
                        # 1/(8*sqrt(m+eps)) = 1/sqrt(ssq + 64*eps)
                        if isq:
                            nc.scalar.activation(ssg[:], ssg[:], AF.Sqrt,
                                                 bias=eps64_t[:])
                        else:
                            nc.scalar.activation(
                                ssg[:], ssg[:], AF.Sqrt, scale=1.0 / HD,
                                bias=eps_t[:]
                            )
                        rsg = scr.tile([P, NH, 1], F32, tag="rsg")
                        nc.vector.reciprocal(rsg[:], ssg[:])
                        tn = scr.tile([P, D], BF16,
                                      tag="tn_q" if isq else "tn_k",
                                      name=f"tn_{i}_{isq}")
                        nc.gpsimd.tensor_mul(
                            tn[:].rearrange("p (h d) -> p h d", d=HD),
                            t[:].rearrange("p (h d) -> p h d", d=HD),
                            rsg[:].to_broadcast((P, NH, HD)),
                        )
                        tn_pair.append(tn)
                    tn_prev = tn_pair

        # ======================= phase C: attention =======================
        with (
            tc.tile_pool(name="oT_pool", bufs=1) as oTp,
            tc.tile_pool(name="attnT_pool", bufs=1) as attnp,
            tc.tile_pool(name="expT", bufs=32) as expp,
            tc.tile_pool(name="rs", bufs=1) as rsp,
            nc.allow_low_precision(reason="softmax in bf16"),
        ):
            oT_all = oTp.tile([P, CT, D], F8, tag="oT_all", name="oT_all")
            nc.sync.dma_start(
                out=oT_all[:], in_=oT_d[:].rearrange("(c p) e -> p c e", p=P)
            )
            attnT = [attnp.tile([P, L], BF16, tag=f"attnT{c}", name=f"attnT{c}")
                     for c in range(CT)]
            # normalized attention in fp8, ct-major for DoubleRow o_proj
            attnT8 = attnp.tile([P, CT, L], F8, tag="attnT8", name="attnT8")
            # 1/rowsum per head-in-pair, [1, NP, L] each (partition 0)
            rsinv = [rsp.tile([1, NP, L], BF16, tag=f"rsinv{hh}", name=f"rsinv{hh}")
                     for hh in range(2)]

            ets = {}  # (hp, hh, jt) -> exp tile [P, L] bf16

            def norm(hp):
                # broadcast 1/rowsum over the 64 chans of each head (K=1 PE
                # matmuls) and scale attnT; bc borrows a slot of the sc ring
                bc = ps_sc.tile([P, L], F32, tag="sc", name=f"bc_{hp}")
                for ic in range(2):
                    for hh in range(2):
                        nc.tensor.matmul(
                            bc[:, ts(ic, 512)], selp[hh][:],
                            rsinv[hh][:, hp, ts(ic, 512)],
                            start=(hh == 0), stop=(hh == 1),
                        )
                nc.vector.tensor_mul(attnT8[:, hp, :], attnT[hp][:], bc[:])

            # software-pipelined over head pairs: scores+exp(hp) interleaved
            # with the AV matmuls of hp-1 at jt granularity (keeps PE busy
            # while exps drain the sc ring), normalization trails by 2.
            with (
                tc.tile_pool(name="psC_sc", bufs=3, space="PSUM") as ps_sc,
                tc.tile_pool(name="psC_av", bufs=1, space="PSUM") as ps_av,
            ):
              for hp in range(NP + 2):
                do_sc = hp < NP
                do_av = 1 <= hp <= NP
                avt = {}

                def evict(f):
                    # attn rows to ScalarE; 1/rowsum via Schraudolph bits
                    # trick (DVE): bits(1/x) ~ RCP_K - hi16(f32 x), reading
                    # the high half-words of the PSUM f32 row directly
                    hh, ic = f // 2, f % 2
                    off = HD * hh
                    a = avt[f]
                    nc.scalar.copy(
                        attnT[hp - 1][off:off + HD, ts(ic, 512)], a[0:HD, :]
                    )
                    hi16 = a[HD:HD + 1, :].bitcast(I16).rearrange(
                        "p (n two) -> p n two", two=2
                    )[:, :, 1]
                    nc.vector.tensor_scalar(
                        rsinv[hh][:, hp - 1, ts(ic, 512)].bitcast(I16),
                        hi16, -1.0, RCP_K, op0=ALU.mult, op1=ALU.add,
                    )

                for jt in range(TQ):
                    if do_av:
                        # AV group f=(hh,ic) runs its 8 chunk-mms in jts
                        # [2f, 2f+1] and is evicted at jt 2f+2, so only two
                        # AV tiles are ever live (PSUM budget) and the four
                        # evictions spread through the step
                        f = jt // 2
                        hh, ic = f // 2, f % 2
                        h = 2 * (hp - 1) + hh
                        if jt % 2 == 0:
                            avt[f] = ps_av.tile([HD + 1, 512], F32,
                                                tag=f"av{f % 2}",
                                                name=f"av_{hp - 1}_{f}")
                        for dj in range(4):
                            ajt = (jt % 2) * 4 + dj
                            nc.tensor.matmul(
                                avt[f][:],
                                v_sb[ajt][:, h, :],
                                ets[(hp - 1, hh, ajt)][:, ts(ic, 512)],
                                start=(ajt == 0), stop=(ajt == TQ - 1),
                            )
                    if do_sc:
                        sc = [ps_sc.tile([P, L], F32, tag="sc",
                                         name=f"sc_{hp}_{jt}_{hh}")
                              for hh in range(2)]
                        for ic in range(2):
                            for hh in range(2):
                                off = HD * hh
                                nc.tensor.matmul(
                                    sc[hh][:, ts(ic, 512)],
                                    knT[hp][off:off + HD, ts(jt, P)],
                                    qnT[hp][off:off + HD, ts(ic, 512)],
                                    start=True, stop=True,
                                    tile_position=(off, 0),
                                )
                        for hh in range(2):
                            e = expp.tile([P, L], BF16, tag="ets",
                                          name=f"ets_{hp}_{hh}_{jt}")
                            ets[(hp, hh, jt)] = e
                            # static split tuned so ScalarE/DVE finish together
                            on_scalar = (hh == 0) or (jt == 4)
                            if on_scalar:
                                nc.scalar.activation(e[:], sc[hh][:], AF.Exp)
                            else:
                                nc.vector.tensor_scalar(
                                    e[:].bitcast(I16), sc[hh][:],
                                    EXP_K16, EXP_B16, op0=ALU.mult, op1=ALU.add,
                                )
                    if do_av and jt >= 2 and jt % 2 == 0:
                        evict(jt // 2 - 1)
                if do_av:
                    evict(3)
                if hp >= 2:
                    norm(hp - 2)

            # =================== phase D: o_proj + residual ===================
            with (
                tc.tile_pool(name="psD", bufs=3, space="PSUM") as psD,
                tc.tile_pool(name="outp", bufs=2) as outp,
            ):
                for i in range(TQ):
                    o0 = psD.tile([P, 512], F32, tag="o0")
                    o1 = psD.tile([P, 256], F32, tag="o1")
                    for cp in range(CT // 2):
                        lhsT = attnT8[:, 2 * cp:2 * cp + 2, ts(i, P)]
                        nc.tensor.matmul(
                            o0[:], lhsT, oT_all[:, 2 * cp:2 * cp + 2, 0:512],
                            start=(cp == 0), stop=(cp == CT // 2 - 1),
                            perf_mode=DR,
                        )
                        nc.tensor.matmul(
                            o1[:], lhsT, oT_all[:, 2 * cp:2 * cp + 2, 512:D],
                            start=(cp == 0), stop=(cp == CT // 2 - 1),
                            perf_mode=DR,
                        )
                    out_sb = outp.tile([P, D], F32, tag="out_sb")
                    nc.vector.scalar_tensor_tensor(
                        out_sb[:, 0:512], o0[:], 1.0 / O_SCALE,
                        x_all[:, i, 0:512], op0=ALU.mult, op1=ALU.add,
                    )
                    nc.vector.scalar_tensor_tensor(
                        out_sb[:, 512:D], o1[:], 1.0 / O_SCALE,
                        x_all[:, i, 512:D], op0=ALU.mult, op1=ALU.add,
                    )
                    nc.sync.dma_start(out=out_d[ts(i, P), :], in_=out_sb[:])

    nc.compile()
    return nc


_NC = None


def _get_nc():
    global _NC
    if _NC is None:
        _NC = build_bass()
    return _NC


def make_in_maps(input_NHWD, qkv_weight, o_weight, o_scale):
    N = input_NHWD.shape[0]
    f8 = ml_dtypes.float8_e4m3
    wT = np.ascontiguousarray(
        qkv_weight.reshape(3 * D, D).T.astype(np.float32) * W_SCALE
    ).astype(f8)
    oT = np.ascontiguousarray(
        (o_weight * o_scale[:, None]).T.astype(np.float32) * O_SCALE
    ).astype(f8)
    in_maps = []
    for i in range(N):
        xi = np.ascontiguousarray(input_NHWD[i].reshape(L, D).astype(np.float32))
        in_maps.append(
            {"x": xi, "xT": np.ascontiguousarray(xi.T).astype(f8),
             "wT": wT, "oT": oT}
        )
    return in_maps


def kernel(input_NHWD, qkv_weight, o_weight, o_scale):
    import time
    from concourse.bass_utils import run_bass_kernel_spmd

    input_NHWD = np.asarray(input_NHWD)
    N, H, W, _ = input_NHWD.shape
    nc = _get_nc()
    in_maps = make_in_maps(np.asarray(input_NHWD), np.asarray(qkv_weight),
                           np.asarray(o_weight), np.asarray(o_scale))
    last_err = None
    for attempt in range(3):
        try:
            res = run_bass_kernel_spmd(nc, in_maps, list(range(N)))
            out = np.stack([res.results[i]["out"] for i in range(N)], axis=0)
            return out.reshape(N, H, W, D).astype(np.float32)
        except Exception as e:  # transient device wedge: clear + retry
            last_err = e
            try:
                import jax
                jax.clear_caches()
                jax.clear_backends()
            except Exception:
                pass
            time.sleep(10)
    raise last_err
